# revision 25
# baseline (speedup 1.0000x reference)
"""ExemplarAttention Trainium2 kernel (8 NeuronCores, exemplar-sharded, transposed).

logits[b,c] = gamma * log(sum_{n:label[n]=c} exp(-beta * sum_k w_k (x[b,k]-e[n,k])^2) + eps)

Strategy (v7):
  - Shard the EXEMPLARS across the 8 cores (N_LOC = 2048 each); every core
    keeps the full batch B=1024.  Per-core HBM traffic is ~1.6MB, so the
    kernel is compute-paced, not DMA-paced (the v1 baseline replicated the
    8.4MB exemplar bank to every core).
  - TRANSPOSED gemm: exemplars on the PSUM partition axis, batch on the
    free axis.  cross_T[n, b] = sum_k e[n,k] * (S*x[b,k]*w[k]) via fp8
    DoubleRow matmuls (weights = e-tile, moving = xw, so each 128-exemplar
    weight tile amortizes over the 1024-wide batch).
  - The exp argument needs no bias at all:
      sim[n,b] = exp(-beta*(x2w[b] + e2w[n] - 2 cross)) factors as
      exp(-beta*x2w[b]) * [E_n * exp(2 beta cross)]
    with E_n = exp(-beta*e2w[n]) folded into HOST-prepared one-hot matmul
    weights ohE[n, c] = E_n * (label[n] == c), and the exp(-beta*x2w[b])
    factor applied by the host during the unshard.
  - Class sums are tiny M=10 PE matmuls over the partition axis.  The exp
    results are written as fp8 PAIRS (two exemplar tiles interleaved in one
    sbuf tile) so each one-hot matmul contracts K=256 in DoubleRow mode:
    the sim matrix is re-streamed through the PE only once at half rate
    (16 matmuls), and VectorE does almost nothing (the v2 design died on a
    28us 80-instruction tensor_reduce stream).
  - Host: partial class sums from the 8 cores are summed in the unshard
    (a 40KB-per-core DMA; collectives cost ~70us under this runtime) and
    log/gamma applied there.  fp8 rounding of sim/ohE is averaged down by
    the ~1600-element class sums (adds ~5e-4 relative error, gate is 2e-2).
"""

import os
from contextlib import ExitStack

import numpy as np

B, N, D, C = 1024, 16384, 512, 10
NCORES = 8
N_LOC = N // NCORES          # 2048 exemplars per core
N_TILES = N_LOC // 128       # 16 exemplar tiles of 128
N_PAIRS = N_TILES // 2       # exp-output pairs (K=256 one-hot DoubleRow)
NG = 2                       # DoubleRow groups (K=256 each)
HALF = 512                   # matmul moving-operand cap (1 psum bank of f32)
EPS = 1e-9
S_SCALE = 128.0              # fp8 scale applied to x*w
N_WARMUP_MM = 6              # HAM warmup matmuls before the main stream
CP = 16                      # class dim padded to 16 (DR ldweights needs
                             # a 16-byte-aligned interleave stride)

_prog_cache = {}


def _np_dt(mybir, name):
    return mybir.dt.np(getattr(mybir.dt, name))


def _build_program(act_scale):
    import concourse.bass as bass  # noqa: F401
    import concourse.tile as tile
    from concourse import bacc, mybir

    fp8 = mybir.dt.float8e4
    bf16 = mybir.dt.bfloat16
    f32 = mybir.dt.float32

    nc = bacc.Bacc("TRN2", target_bir_lowering=False, debug=False,
                   num_devices=NCORES)

    et_d = nc.dram_tensor("et", [128, N_TILES, NG, 2, 128], fp8,
                          kind="ExternalInput").ap()
    xw_d = nc.dram_tensor("xw", [128, NG, 2, B], fp8,
                          kind="ExternalInput").ap()
    ohe_d = nc.dram_tensor("ohe", [128, N_PAIRS, 2, CP], fp8,
                           kind="ExternalInput").ap()
    out_d = nc.dram_tensor("parts", [C, B], f32, kind="ExternalOutput").ap()

    with tile.TileContext(nc) as tc, ExitStack() as ctx:
        singles = ctx.enter_context(tc.tile_pool(name="singles", bufs=1))
        ct_pool = ctx.enter_context(tc.tile_pool(name="ct", bufs=3,
                                                 space="PSUM"))
        cls_pool = ctx.enter_context(tc.tile_pool(name="cls", bufs=1,
                                                  space="PSUM"))
        sc_pool = ctx.enter_context(tc.tile_pool(name="sc", bufs=4))

        # Warmup operands from a memset tile (no DMA dependency): opens the
        # HAM clock gate while the DMA streams land.
        dmy = singles.tile([128, 128 + HALF], bf16)
        nc.vector.memset(dmy[:, :], 0.0)

        # Dummy activation so the ACT table load runs during the DMA window.
        dummy = singles.tile([128, 1], f32)
        nc.vector.memset(dummy[:, :], 0.0)
        nc.scalar.activation(out=dummy[:, :], in_=dummy[:, :],
                             func=mybir.ActivationFunctionType.Exp, scale=1.0)

        # All rings share the 16 HW DMA engines; head-critical chunks go
        # first on both rings, bulk strictly behind so it cannot starve them.
        et_sb = singles.tile([128, N_TILES, NG, 2, 128], fp8)
        xw_sb = singles.tile([128, NG, 2, B], fp8)
        ohe_sb = singles.tile([128, N_PAIRS, 2, CP], fp8)
        # Ring 1 (sync) carries the head-critical bytes in exact consumption
        # order of the h-split first pair: the first exp needs only 384KB.
        nc.sync.dma_start(out=et_sb[:, 0:2], in_=et_d[:, 0:2])
        nc.sync.dma_start(out=xw_sb[:, 0:1, :, 0:HALF],
                          in_=xw_d[:, 0:1, :, 0:HALF])
        nc.sync.dma_start(out=xw_sb[:, 1:NG, :, 0:HALF],
                          in_=xw_d[:, 1:NG, :, 0:HALF])
        nc.sync.dma_start(out=xw_sb[:, 0:1, :, HALF:B],
                          in_=xw_d[:, 0:1, :, HALF:B])
        nc.sync.dma_start(out=xw_sb[:, 1:NG, :, HALF:B],
                          in_=xw_d[:, 1:NG, :, HALF:B])
        nc.scalar.dma_start(out=et_sb[:, 2:6], in_=et_d[:, 2:6])
        nc.scalar.dma_start(out=et_sb[:, 6:10], in_=et_d[:, 6:10])
        nc.scalar.dma_start(out=ohe_sb[:, :], in_=ohe_d[:, :])
        nc.scalar.dma_start(out=et_sb[:, 10:N_TILES], in_=et_d[:, 10:N_TILES])

        cls_ps = cls_pool.tile([128, B], f32)

        ct0 = ct_pool.tile([128, B], f32, tag="ct", name="ct0")
        for _ in range(N_WARMUP_MM):
            nc.tensor.matmul(ct0[:, 0:HALF], lhsT=dmy[:, 0:128],
                             rhs=dmy[:, 128:], start=True, stop=True)

        sc_pairs = [None] * N_PAIRS

        def emit_onehot(p):
            # One K=256 DoubleRow matmul per b-half: contracts both tiles of
            # the pair in a single pass over the sim values.
            rhs3 = sc_pairs[p].rearrange("q (k b) -> q k b", k=2)
            for h in range(B // HALF):
                cs = slice(h * HALF, (h + 1) * HALF)
                nc.tensor.matmul(
                    cls_ps[0:CP, cs],
                    lhsT=ohe_sb[:, p, :, :],
                    rhs=rhs3[:, :, cs],
                    start=(p == 0), stop=(p == N_PAIRS - 1),
                    perf_mode=mybir.MatmulPerfMode.DoubleRow)

        for n in range(N_TILES):
            ct = ct0 if n == 0 else ct_pool.tile([128, B], f32, tag="ct",
                                                 name=f"ct{n}")
            p, s = divmod(n, 2)
            if s == 0:
                sc_pairs[p] = sc_pool.tile([128, 2 * B], fp8, tag="sc",
                                           name=f"scp{p}")
            if n < 2:
                # h-split head: matmuls h-major and exp per b-half, so the
                # ACT stream starts as soon as the h0 operands land instead
                # of waiting for the full critical DMA prefix.
                for h in range(B // HALF):
                    cs = slice(h * HALF, (h + 1) * HALF)
                    for g in range(NG):
                        nc.tensor.matmul(
                            ct[:, cs], lhsT=et_sb[:, n, g, :, :],
                            rhs=xw_sb[:, g, :, cs], start=(g == 0),
                            stop=(g == NG - 1),
                            perf_mode=mybir.MatmulPerfMode.DoubleRow)
                    nc.scalar.activation(
                        out=sc_pairs[p][:, s * B + h * HALF:
                                        s * B + (h + 1) * HALF],
                        in_=ct[:, cs],
                        func=mybir.ActivationFunctionType.Exp,
                        scale=act_scale)
                continue
            for g in range(NG):
                for h in range(B // HALF):
                    cs = slice(h * HALF, (h + 1) * HALF)
                    nc.tensor.matmul(
                        ct[:, cs], lhsT=et_sb[:, n, g, :, :],
                        rhs=xw_sb[:, g, :, cs], start=(g == 0),
                        stop=(g == NG - 1),
                        perf_mode=mybir.MatmulPerfMode.DoubleRow)
            nc.scalar.activation(
                out=sc_pairs[p][:, s * B:(s + 1) * B], in_=ct[:, :],
                func=mybir.ActivationFunctionType.Exp, scale=act_scale)
            # one-hot matmuls trail by one pair so the PE never queue-blocks
            # on an exp that hasn't retired yet
            if s == 1 and p >= 1:
                emit_onehot(p - 1)
        emit_onehot(N_PAIRS - 1)

        # DMA cannot read PSUM; bounce the class rows through SBUF on the
        # otherwise-idle VectorE.
        cls_sb = singles.tile([128, B], f32)
        nc.vector.tensor_copy(cls_sb[0:C, :], cls_ps[0:C, :])
        nc.sync.dma_start(out=out_d[:, :], in_=cls_sb[0:C, :])

    nc.compile()
    return nc


def _prepare(x, ex_feats, ex_labels, w_unconstrained, gamma_unconstrained,
             beta_unconstrained):
    from concourse import mybir

    x = np.asarray(x, dtype=np.float64)
    e = np.asarray(ex_feats, dtype=np.float64)
    labels = np.asarray(ex_labels).astype(np.int64)
    wu = np.asarray(w_unconstrained, dtype=np.float64)

    beta = float(np.log1p(np.exp(np.float64(beta_unconstrained)))) + EPS
    gamma = float(np.log1p(np.exp(np.float64(gamma_unconstrained)))) + EPS
    wexp = np.exp(wu - wu.max())
    w = wexp / wexp.sum() + EPS

    bf16 = _np_dt(mybir, "bfloat16")  # noqa: F841
    fp8 = _np_dt(mybir, "float8e4")

    xw = x * w[None, :]                               # (B, D)
    x2w = (x * x) @ w                                 # (B,)
    e2w = (e * e) @ w                                 # (N,)
    E = np.exp(-beta * e2w)                           # (N,) per-exemplar wt

    # xw_sb[p, g, s, b] = S * xw[b, (2g+s)*128 + p]
    xw_t = np.ascontiguousarray(
        (S_SCALE * xw).T.reshape(NG, 2, 128, B).transpose(2, 0, 1, 3)
    ).astype(fp8)

    onehot = (labels[:, None] == np.arange(C)[None, :])  # (N, C)
    ohE_full = (onehot * E[:, None])                     # (N, C) f64

    per_core = []
    for cid in range(NCORES):
        rows = slice(cid * N_LOC, (cid + 1) * N_LOC)
        # et[p, n, g, s, m] = e[cid*N_LOC + n*128 + m, (2g+s)*128 + p]
        et = np.ascontiguousarray(
            e[rows].T.reshape(NG, 2, 128, N_TILES, 128)
            .transpose(2, 3, 0, 1, 4)).astype(fp8)
        # ohe[r, pair, s, c] = ohE_full[cid*N_LOC + (2*pair+s)*128 + r, c]
        ohe = np.zeros((128, N_PAIRS, 2, CP), dtype=fp8)
        ohe[:, :, :, 0:C] = (
            ohE_full[rows].reshape(N_PAIRS, 2, 128, C).transpose(2, 0, 1, 3)
        ).astype(fp8)
        per_core.append({
            "et": et,
            "xw": xw_t,
            "ohe": ohe,
        })
    return per_core, beta, gamma, x2w


def kernel(x, ex_feats, ex_labels, w_unconstrained, gamma_unconstrained,
           beta_unconstrained, _want_results=False, **run_kwargs):
    from concourse.bass_utils import run_bass_kernel_spmd

    per_core, beta, gamma, x2w = _prepare(
        x, ex_feats, ex_labels, w_unconstrained, gamma_unconstrained,
        beta_unconstrained)

    act_scale = float(2.0 * beta / S_SCALE)
    key = round(act_scale, 14)
    if key not in _prog_cache:
        _prog_cache[key] = _build_program(act_scale)
    nc = _prog_cache[key]

    res = run_bass_kernel_spmd(nc, per_core, list(range(NCORES)), **run_kwargs)

    device_sum = np.zeros((C, B), dtype=np.float64)
    for cid in range(NCORES):
        device_sum += np.asarray(res.results[cid]["parts"], dtype=np.float64)
    class_sum = device_sum.T * np.exp(-beta * x2w)[:, None]   # (B, C)
    out = (gamma * np.log(class_sum + EPS)).astype(np.float32)
    if _want_results:
        return out, res
    return out


# revision 26
# speedup vs baseline: 1.0417x; 1.0417x over previous
"""ExemplarAttention Trainium2 kernel (8 NeuronCores, exemplar-sharded, transposed).

logits[b,c] = gamma * log(sum_{n:label[n]=c} exp(-beta * sum_k w_k (x[b,k]-e[n,k])^2) + eps)

Strategy (v7):
  - Shard the EXEMPLARS across the 8 cores (N_LOC = 2048 each); every core
    keeps the full batch B=1024.  Per-core HBM traffic is ~1.6MB, so the
    kernel is compute-paced, not DMA-paced (the v1 baseline replicated the
    8.4MB exemplar bank to every core).
  - TRANSPOSED gemm: exemplars on the PSUM partition axis, batch on the
    free axis.  cross_T[n, b] = sum_k e[n,k] * (S*x[b,k]*w[k]) via fp8
    DoubleRow matmuls (weights = e-tile, moving = xw, so each 128-exemplar
    weight tile amortizes over the 1024-wide batch).
  - The exp argument needs no bias at all:
      sim[n,b] = exp(-beta*(x2w[b] + e2w[n] - 2 cross)) factors as
      exp(-beta*x2w[b]) * [E_n * exp(2 beta cross)]
    with E_n = exp(-beta*e2w[n]) folded into HOST-prepared one-hot matmul
    weights ohE[n, c] = E_n * (label[n] == c), and the exp(-beta*x2w[b])
    factor applied by the host during the unshard.
  - Class sums are tiny M=10 PE matmuls over the partition axis.  The exp
    results are written as fp8 PAIRS (two exemplar tiles interleaved in one
    sbuf tile) so each one-hot matmul contracts K=256 in DoubleRow mode:
    the sim matrix is re-streamed through the PE only once at half rate
    (16 matmuls), and VectorE does almost nothing (the v2 design died on a
    28us 80-instruction tensor_reduce stream).
  - Host: partial class sums from the 8 cores are summed in the unshard
    (a 40KB-per-core DMA; collectives cost ~70us under this runtime) and
    log/gamma applied there.  fp8 rounding of sim/ohE is averaged down by
    the ~1600-element class sums (adds ~5e-4 relative error, gate is 2e-2).
"""

import os
from contextlib import ExitStack

import numpy as np

B, N, D, C = 1024, 16384, 512, 10
NCORES = 8
N_LOC = N // NCORES          # 2048 exemplars per core
N_TILES = N_LOC // 128       # 16 exemplar tiles of 128
N_PAIRS = N_TILES // 2       # exp-output pairs (K=256 one-hot DoubleRow)
NG = 2                       # DoubleRow groups (K=256 each)
HALF = 512                   # matmul moving-operand cap (1 psum bank of f32)
EPS = 1e-9
S_SCALE = 128.0              # fp8 scale applied to x*w
N_WARMUP_MM = 6              # HAM warmup matmuls before the main stream
CP = 16                      # class dim padded to 16 (DR ldweights needs
                             # a 16-byte-aligned interleave stride)

_prog_cache = {}


def _np_dt(mybir, name):
    return mybir.dt.np(getattr(mybir.dt, name))


def _build_program(act_scale):
    import concourse.bass as bass  # noqa: F401
    import concourse.tile as tile
    from concourse import bacc, mybir

    fp8 = mybir.dt.float8e4
    bf16 = mybir.dt.bfloat16
    f32 = mybir.dt.float32

    nc = bacc.Bacc("TRN2", target_bir_lowering=False, debug=False,
                   num_devices=NCORES)

    et_d = nc.dram_tensor("et", [128, N_TILES, NG, 2, 128], fp8,
                          kind="ExternalInput").ap()
    xw_d = nc.dram_tensor("xw", [128, NG, 2, B], fp8,
                          kind="ExternalInput").ap()
    ohe_d = nc.dram_tensor("ohe", [128, N_PAIRS, 2, CP], fp8,
                           kind="ExternalInput").ap()
    out_d = nc.dram_tensor("parts", [C, B], f32, kind="ExternalOutput").ap()

    with tile.TileContext(nc) as tc, ExitStack() as ctx:
        singles = ctx.enter_context(tc.tile_pool(name="singles", bufs=1))
        ct_pool = ctx.enter_context(tc.tile_pool(name="ct", bufs=3,
                                                 space="PSUM"))
        cls_pool = ctx.enter_context(tc.tile_pool(name="cls", bufs=1,
                                                  space="PSUM"))
        sc_pool = ctx.enter_context(tc.tile_pool(name="sc", bufs=4))

        # Warmup operands from a memset tile (no DMA dependency): opens the
        # HAM clock gate while the DMA streams land.
        dmy = singles.tile([128, 128 + HALF], bf16)
        nc.vector.memset(dmy[:, :], 0.0)

        # Dummy activation so the ACT table load runs during the DMA window.
        dummy = singles.tile([128, 1], f32)
        nc.vector.memset(dummy[:, :], 0.0)
        nc.scalar.activation(out=dummy[:, :], in_=dummy[:, :],
                             func=mybir.ActivationFunctionType.Exp, scale=1.0)

        # All rings share the 16 HW DMA engines; head-critical chunks go
        # first on both rings, bulk strictly behind so it cannot starve them.
        et_sb = singles.tile([128, N_TILES, NG, 2, 128], fp8)
        xw_sb = singles.tile([128, NG, 2, B], fp8)
        ohe_sb = singles.tile([128, N_PAIRS, 2, CP], fp8)
        # Ring 1 (sync) carries the head-critical bytes in exact consumption
        # order of the h-split first pair: the first exp needs only 384KB.
        nc.sync.dma_start(out=et_sb[:, 0:2], in_=et_d[:, 0:2])
        nc.sync.dma_start(out=xw_sb[:, 0:1, :, 0:HALF],
                          in_=xw_d[:, 0:1, :, 0:HALF])
        nc.sync.dma_start(out=xw_sb[:, 1:NG, :, 0:HALF],
                          in_=xw_d[:, 1:NG, :, 0:HALF])
        nc.sync.dma_start(out=xw_sb[:, 0:1, :, HALF:B],
                          in_=xw_d[:, 0:1, :, HALF:B])
        nc.sync.dma_start(out=xw_sb[:, 1:NG, :, HALF:B],
                          in_=xw_d[:, 1:NG, :, HALF:B])
        nc.sync.dma_start(out=et_sb[:, 2:6], in_=et_d[:, 2:6])
        nc.sync.dma_start(out=et_sb[:, 6:10], in_=et_d[:, 6:10])
        nc.sync.dma_start(out=ohe_sb[:, :], in_=ohe_d[:, :])
        nc.sync.dma_start(out=et_sb[:, 10:N_TILES], in_=et_d[:, 10:N_TILES])

        cls_ps = cls_pool.tile([128, B], f32)

        ct0 = ct_pool.tile([128, B], f32, tag="ct", name="ct0")
        for _ in range(N_WARMUP_MM):
            nc.tensor.matmul(ct0[:, 0:HALF], lhsT=dmy[:, 0:128],
                             rhs=dmy[:, 128:], start=True, stop=True)

        sc_pairs = [None] * N_PAIRS

        def emit_onehot(p):
            # One K=256 DoubleRow matmul per b-half: contracts both tiles of
            # the pair in a single pass over the sim values.
            rhs3 = sc_pairs[p].rearrange("q (k b) -> q k b", k=2)
            for h in range(B // HALF):
                cs = slice(h * HALF, (h + 1) * HALF)
                nc.tensor.matmul(
                    cls_ps[0:CP, cs],
                    lhsT=ohe_sb[:, p, :, :],
                    rhs=rhs3[:, :, cs],
                    start=(p == 0), stop=(p == N_PAIRS - 1),
                    perf_mode=mybir.MatmulPerfMode.DoubleRow)

        for n in range(N_TILES):
            ct = ct0 if n == 0 else ct_pool.tile([128, B], f32, tag="ct",
                                                 name=f"ct{n}")
            p, s = divmod(n, 2)
            if s == 0:
                sc_pairs[p] = sc_pool.tile([128, 2 * B], fp8, tag="sc",
                                           name=f"scp{p}")
            if n < 2:
                # h-split head: matmuls h-major and exp per b-half, so the
                # ACT stream starts as soon as the h0 operands land instead
                # of waiting for the full critical DMA prefix.
                for h in range(B // HALF):
                    cs = slice(h * HALF, (h + 1) * HALF)
                    for g in range(NG):
                        nc.tensor.matmul(
                            ct[:, cs], lhsT=et_sb[:, n, g, :, :],
                            rhs=xw_sb[:, g, :, cs], start=(g == 0),
                            stop=(g == NG - 1),
                            perf_mode=mybir.MatmulPerfMode.DoubleRow)
                    nc.scalar.activation(
                        out=sc_pairs[p][:, s * B + h * HALF:
                                        s * B + (h + 1) * HALF],
                        in_=ct[:, cs],
                        func=mybir.ActivationFunctionType.Exp,
                        scale=act_scale)
                continue
            for g in range(NG):
                for h in range(B // HALF):
                    cs = slice(h * HALF, (h + 1) * HALF)
                    nc.tensor.matmul(
                        ct[:, cs], lhsT=et_sb[:, n, g, :, :],
                        rhs=xw_sb[:, g, :, cs], start=(g == 0),
                        stop=(g == NG - 1),
                        perf_mode=mybir.MatmulPerfMode.DoubleRow)
            nc.scalar.activation(
                out=sc_pairs[p][:, s * B:(s + 1) * B], in_=ct[:, :],
                func=mybir.ActivationFunctionType.Exp, scale=act_scale)
            # one-hot matmuls trail by one pair so the PE never queue-blocks
            # on an exp that hasn't retired yet
            if s == 1 and p >= 1:
                emit_onehot(p - 1)
        emit_onehot(N_PAIRS - 1)

        # DMA cannot read PSUM; bounce the class rows through SBUF on the
        # otherwise-idle VectorE.
        cls_sb = singles.tile([128, B], f32)
        nc.vector.tensor_copy(cls_sb[0:C, :], cls_ps[0:C, :])
        nc.sync.dma_start(out=out_d[:, :], in_=cls_sb[0:C, :])

    nc.compile()
    return nc


def _prepare(x, ex_feats, ex_labels, w_unconstrained, gamma_unconstrained,
             beta_unconstrained):
    from concourse import mybir

    x = np.asarray(x, dtype=np.float64)
    e = np.asarray(ex_feats, dtype=np.float64)
    labels = np.asarray(ex_labels).astype(np.int64)
    wu = np.asarray(w_unconstrained, dtype=np.float64)

    beta = float(np.log1p(np.exp(np.float64(beta_unconstrained)))) + EPS
    gamma = float(np.log1p(np.exp(np.float64(gamma_unconstrained)))) + EPS
    wexp = np.exp(wu - wu.max())
    w = wexp / wexp.sum() + EPS

    bf16 = _np_dt(mybir, "bfloat16")  # noqa: F841
    fp8 = _np_dt(mybir, "float8e4")

    xw = x * w[None, :]                               # (B, D)
    x2w = (x * x) @ w                                 # (B,)
    e2w = (e * e) @ w                                 # (N,)
    E = np.exp(-beta * e2w)                           # (N,) per-exemplar wt

    # xw_sb[p, g, s, b] = S * xw[b, (2g+s)*128 + p]
    xw_t = np.ascontiguousarray(
        (S_SCALE * xw).T.reshape(NG, 2, 128, B).transpose(2, 0, 1, 3)
    ).astype(fp8)

    onehot = (labels[:, None] == np.arange(C)[None, :])  # (N, C)
    ohE_full = (onehot * E[:, None])                     # (N, C) f64

    per_core = []
    for cid in range(NCORES):
        rows = slice(cid * N_LOC, (cid + 1) * N_LOC)
        # et[p, n, g, s, m] = e[cid*N_LOC + n*128 + m, (2g+s)*128 + p]
        et = np.ascontiguousarray(
            e[rows].T.reshape(NG, 2, 128, N_TILES, 128)
            .transpose(2, 3, 0, 1, 4)).astype(fp8)
        # ohe[r, pair, s, c] = ohE_full[cid*N_LOC + (2*pair+s)*128 + r, c]
        ohe = np.zeros((128, N_PAIRS, 2, CP), dtype=fp8)
        ohe[:, :, :, 0:C] = (
            ohE_full[rows].reshape(N_PAIRS, 2, 128, C).transpose(2, 0, 1, 3)
        ).astype(fp8)
        per_core.append({
            "et": et,
            "xw": xw_t,
            "ohe": ohe,
        })
    return per_core, beta, gamma, x2w


def kernel(x, ex_feats, ex_labels, w_unconstrained, gamma_unconstrained,
           beta_unconstrained, _want_results=False, **run_kwargs):
    from concourse.bass_utils import run_bass_kernel_spmd

    per_core, beta, gamma, x2w = _prepare(
        x, ex_feats, ex_labels, w_unconstrained, gamma_unconstrained,
        beta_unconstrained)

    act_scale = float(2.0 * beta / S_SCALE)
    key = round(act_scale, 14)
    if key not in _prog_cache:
        _prog_cache[key] = _build_program(act_scale)
    nc = _prog_cache[key]

    res = run_bass_kernel_spmd(nc, per_core, list(range(NCORES)), **run_kwargs)

    device_sum = np.zeros((C, B), dtype=np.float64)
    for cid in range(NCORES):
        device_sum += np.asarray(res.results[cid]["parts"], dtype=np.float64)
    class_sum = device_sum.T * np.exp(-beta * x2w)[:, None]   # (B, C)
    out = (gamma * np.log(class_sum + EPS)).astype(np.float32)
    if _want_results:
        return out, res
    return out


# revision 27
# speedup vs baseline: 1.0759x; 1.0329x over previous
"""ExemplarAttention Trainium2 kernel (8 NeuronCores, exemplar-sharded, transposed).

logits[b,c] = gamma * log(sum_{n:label[n]=c} exp(-beta * sum_k w_k (x[b,k]-e[n,k])^2) + eps)

Strategy (v7):
  - Shard the EXEMPLARS across the 8 cores (N_LOC = 2048 each); every core
    keeps the full batch B=1024.  Per-core HBM traffic is ~1.6MB, so the
    kernel is compute-paced, not DMA-paced (the v1 baseline replicated the
    8.4MB exemplar bank to every core).
  - TRANSPOSED gemm: exemplars on the PSUM partition axis, batch on the
    free axis.  cross_T[n, b] = sum_k e[n,k] * (S*x[b,k]*w[k]) via fp8
    DoubleRow matmuls (weights = e-tile, moving = xw, so each 128-exemplar
    weight tile amortizes over the 1024-wide batch).
  - The exp argument needs no bias at all:
      sim[n,b] = exp(-beta*(x2w[b] + e2w[n] - 2 cross)) factors as
      exp(-beta*x2w[b]) * [E_n * exp(2 beta cross)]
    with E_n = exp(-beta*e2w[n]) folded into HOST-prepared one-hot matmul
    weights ohE[n, c] = E_n * (label[n] == c), and the exp(-beta*x2w[b])
    factor applied by the host during the unshard.
  - Class sums are tiny M=10 PE matmuls over the partition axis.  The exp
    results are written as fp8 PAIRS (two exemplar tiles interleaved in one
    sbuf tile) so each one-hot matmul contracts K=256 in DoubleRow mode:
    the sim matrix is re-streamed through the PE only once at half rate
    (16 matmuls), and VectorE does almost nothing (the v2 design died on a
    28us 80-instruction tensor_reduce stream).
  - Host: partial class sums from the 8 cores are summed in the unshard
    (a 40KB-per-core DMA; collectives cost ~70us under this runtime) and
    log/gamma applied there.  fp8 rounding of sim/ohE is averaged down by
    the ~1600-element class sums (adds ~5e-4 relative error, gate is 2e-2).
"""

import os
from contextlib import ExitStack

import numpy as np

B, N, D, C = 1024, 16384, 512, 10
NCORES = 8
N_LOC = N // NCORES          # 2048 exemplars per core
N_TILES = N_LOC // 128       # 16 exemplar tiles of 128
N_PAIRS = N_TILES // 2       # exp-output pairs (K=256 one-hot DoubleRow)
NG = 2                       # DoubleRow groups (K=256 each)
HALF = 512                   # matmul moving-operand cap (1 psum bank of f32)
EPS = 1e-9
S_SCALE = 128.0              # fp8 scale applied to x*w
N_WARMUP_MM = 8              # HAM warmup matmuls before the main stream
CP = 16                      # class dim padded to 16 (DR ldweights needs
                             # a 16-byte-aligned interleave stride)

_prog_cache = {}


def _np_dt(mybir, name):
    return mybir.dt.np(getattr(mybir.dt, name))


def _build_program(act_scale):
    import concourse.bass as bass  # noqa: F401
    import concourse.tile as tile
    from concourse import bacc, mybir

    fp8 = mybir.dt.float8e4
    bf16 = mybir.dt.bfloat16
    f32 = mybir.dt.float32

    nc = bacc.Bacc("TRN2", target_bir_lowering=False, debug=False,
                   num_devices=NCORES)

    et_d = nc.dram_tensor("et", [128, N_TILES, NG, 2, 128], fp8,
                          kind="ExternalInput").ap()
    xw_d = nc.dram_tensor("xw", [128, NG, 2, B], fp8,
                          kind="ExternalInput").ap()
    ohe_d = nc.dram_tensor("ohe", [128, N_PAIRS, 2, CP], fp8,
                           kind="ExternalInput").ap()
    out_d = nc.dram_tensor("parts", [C, B], f32, kind="ExternalOutput").ap()

    with tile.TileContext(nc) as tc, ExitStack() as ctx:
        singles = ctx.enter_context(tc.tile_pool(name="singles", bufs=1))
        ct_pool = ctx.enter_context(tc.tile_pool(name="ct", bufs=3,
                                                 space="PSUM"))
        cls_pool = ctx.enter_context(tc.tile_pool(name="cls", bufs=1,
                                                  space="PSUM"))
        sc_pool = ctx.enter_context(tc.tile_pool(name="sc", bufs=4))

        # Warmup operands from a memset tile (no DMA dependency): opens the
        # HAM clock gate while the DMA streams land.
        dmy = singles.tile([128, 128 + HALF], bf16)
        nc.vector.memset(dmy[:, :], 0.0)

        # Dummy activation so the ACT table load runs during the DMA window.
        dummy = singles.tile([128, 1], f32)
        nc.vector.memset(dummy[:, :], 0.0)
        nc.scalar.activation(out=dummy[:, :], in_=dummy[:, :],
                             func=mybir.ActivationFunctionType.Exp, scale=1.0)

        # All rings share the 16 HW DMA engines; head-critical chunks go
        # first on both rings, bulk strictly behind so it cannot starve them.
        et_sb = singles.tile([128, N_TILES, NG, 2, 128], fp8)
        xw_sb = singles.tile([128, NG, 2, B], fp8)
        ohe_sb = singles.tile([128, N_PAIRS, 2, CP], fp8)
        nc.sync.dma_start(out=et_sb[:, 0:2], in_=et_d[:, 0:2])
        nc.scalar.dma_start(out=xw_sb[:, 0:1], in_=xw_d[:, 0:1])
        nc.scalar.dma_start(out=xw_sb[:, 1:NG], in_=xw_d[:, 1:NG])
        nc.sync.dma_start(out=et_sb[:, 2:6], in_=et_d[:, 2:6])
        nc.sync.dma_start(out=et_sb[:, 6:10], in_=et_d[:, 6:10])
        nc.scalar.dma_start(out=ohe_sb[:, :], in_=ohe_d[:, :])
        nc.sync.dma_start(out=et_sb[:, 10:N_TILES], in_=et_d[:, 10:N_TILES])

        cls_ps = cls_pool.tile([128, B], f32)

        ct0 = ct_pool.tile([128, B], f32, tag="ct", name="ct0")
        for _ in range(N_WARMUP_MM):
            nc.tensor.matmul(ct0[:, 0:HALF], lhsT=dmy[:, 0:128],
                             rhs=dmy[:, 128:], start=True, stop=True)

        sc_pairs = [None] * N_PAIRS

        def emit_onehot(p):
            # One K=256 DoubleRow matmul per b-half: contracts both tiles of
            # the pair in a single pass over the sim values.
            rhs3 = sc_pairs[p].rearrange("q (k b) -> q k b", k=2)
            for h in range(B // HALF):
                cs = slice(h * HALF, (h + 1) * HALF)
                nc.tensor.matmul(
                    cls_ps[0:CP, cs],
                    lhsT=ohe_sb[:, p, :, :],
                    rhs=rhs3[:, :, cs],
                    start=(p == 0), stop=(p == N_PAIRS - 1),
                    perf_mode=mybir.MatmulPerfMode.DoubleRow)

        for n in range(N_TILES):
            ct = ct0 if n == 0 else ct_pool.tile([128, B], f32, tag="ct",
                                                 name=f"ct{n}")
            p, s = divmod(n, 2)
            if s == 0:
                sc_pairs[p] = sc_pool.tile([128, 2 * B], fp8, tag="sc",
                                           name=f"scp{p}")
            for g in range(NG):
                for h in range(B // HALF):
                    cs = slice(h * HALF, (h + 1) * HALF)
                    nc.tensor.matmul(
                        ct[:, cs], lhsT=et_sb[:, n, g, :, :],
                        rhs=xw_sb[:, g, :, cs], start=(g == 0),
                        stop=(g == NG - 1),
                        perf_mode=mybir.MatmulPerfMode.DoubleRow)
            nc.scalar.activation(
                out=sc_pairs[p][:, s * B:(s + 1) * B], in_=ct[:, :],
                func=mybir.ActivationFunctionType.Exp, scale=act_scale)
            # one-hot matmuls trail by one pair so the PE never queue-blocks
            # on an exp that hasn't retired yet
            if s == 1 and p >= 1:
                emit_onehot(p - 1)
        emit_onehot(N_PAIRS - 1)

        # DMA cannot read PSUM; bounce the class rows through SBUF on the
        # otherwise-idle VectorE.
        cls_sb = singles.tile([128, B], f32)
        nc.vector.tensor_copy(cls_sb[0:C, :], cls_ps[0:C, :])
        nc.sync.dma_start(out=out_d[:, :], in_=cls_sb[0:C, :])

    nc.compile()
    return nc


def _prepare(x, ex_feats, ex_labels, w_unconstrained, gamma_unconstrained,
             beta_unconstrained):
    from concourse import mybir

    x = np.asarray(x, dtype=np.float64)
    e = np.asarray(ex_feats, dtype=np.float64)
    labels = np.asarray(ex_labels).astype(np.int64)
    wu = np.asarray(w_unconstrained, dtype=np.float64)

    beta = float(np.log1p(np.exp(np.float64(beta_unconstrained)))) + EPS
    gamma = float(np.log1p(np.exp(np.float64(gamma_unconstrained)))) + EPS
    wexp = np.exp(wu - wu.max())
    w = wexp / wexp.sum() + EPS

    bf16 = _np_dt(mybir, "bfloat16")  # noqa: F841
    fp8 = _np_dt(mybir, "float8e4")

    xw = x * w[None, :]                               # (B, D)
    x2w = (x * x) @ w                                 # (B,)
    e2w = (e * e) @ w                                 # (N,)
    E = np.exp(-beta * e2w)                           # (N,) per-exemplar wt

    # xw_sb[p, g, s, b] = S * xw[b, (2g+s)*128 + p]
    xw_t = np.ascontiguousarray(
        (S_SCALE * xw).T.reshape(NG, 2, 128, B).transpose(2, 0, 1, 3)
    ).astype(fp8)

    onehot = (labels[:, None] == np.arange(C)[None, :])  # (N, C)
    ohE_full = (onehot * E[:, None])                     # (N, C) f64

    per_core = []
    for cid in range(NCORES):
        rows = slice(cid * N_LOC, (cid + 1) * N_LOC)
        # et[p, n, g, s, m] = e[cid*N_LOC + n*128 + m, (2g+s)*128 + p]
        et = np.ascontiguousarray(
            e[rows].T.reshape(NG, 2, 128, N_TILES, 128)
            .transpose(2, 3, 0, 1, 4)).astype(fp8)
        # ohe[r, pair, s, c] = ohE_full[cid*N_LOC + (2*pair+s)*128 + r, c]
        ohe = np.zeros((128, N_PAIRS, 2, CP), dtype=fp8)
        ohe[:, :, :, 0:C] = (
            ohE_full[rows].reshape(N_PAIRS, 2, 128, C).transpose(2, 0, 1, 3)
        ).astype(fp8)
        per_core.append({
            "et": et,
            "xw": xw_t,
            "ohe": ohe,
        })
    return per_core, beta, gamma, x2w


def kernel(x, ex_feats, ex_labels, w_unconstrained, gamma_unconstrained,
           beta_unconstrained, _want_results=False, **run_kwargs):
    from concourse.bass_utils import run_bass_kernel_spmd

    per_core, beta, gamma, x2w = _prepare(
        x, ex_feats, ex_labels, w_unconstrained, gamma_unconstrained,
        beta_unconstrained)

    act_scale = float(2.0 * beta / S_SCALE)
    key = round(act_scale, 14)
    if key not in _prog_cache:
        _prog_cache[key] = _build_program(act_scale)
    nc = _prog_cache[key]

    res = run_bass_kernel_spmd(nc, per_core, list(range(NCORES)), **run_kwargs)

    device_sum = np.zeros((C, B), dtype=np.float64)
    for cid in range(NCORES):
        device_sum += np.asarray(res.results[cid]["parts"], dtype=np.float64)
    class_sum = device_sum.T * np.exp(-beta * x2w)[:, None]   # (B, C)
    out = (gamma * np.log(class_sum + EPS)).astype(np.float32)
    if _want_results:
        return out, res
    return out
